# revision 19
# baseline (speedup 1.0000x reference)
"""Causal self-attention (B=4, T=2048, C=1024, H=16, D=64) on 8 trn2 NeuronCores.

Sharding: core = 2*b + g  (b = batch index 0..3, g = head-group 0..1).
Each core handles one batch and 8 heads (head-dim columns g*512..g*512+512):
  - QKV projection for its slice (tensor parallel over heads, data parallel on B)
  - flash-style causal attention in S^T layout (keys on partitions)
  - partial output projection  z_partial = y_heads @ W_proj[rows of its heads]
Host unshard: z[b] = z_partial[2b] + z_partial[2b+1] + b_proj.

Kernel layout notes (per core):
  qkT  [128, 8, T]   : rows 0..511 = Q^T (pre-scaled by 1/sqrt(D)), 512..1023 = K^T
                       head h lives in chunk h//2 (+4 for K), partitions (h%2)*64..+64
  vna  [128, 16, 512]: V in natural [t, d] layout, t-chunks of 128 on partitions
  S^T  computed per (head-pair, query-block ib of 512, key-chunk jb of 128):
       psum[128,2,512] <- two row-tiled matmuls (K=64 each, heads share the array)
       exp on ScalarE (PSUM->SBUF), causal mask as 0/1 multiply on diagonal chunks
  PV   col-tiled pair of matmuls (M=64 each) accumulating over jb
  denominators: M=1 matmuls with an all-ones (or attention-mask) column as lhsT
  normalize: reciprocal + K=1 broadcast matmul + elementwise multiply -> yT
  proj: z^T-free layout, lhsT = yT chunks, rhs = W_proj rows, DMA out
"""

import sys

import numpy as np

if "/opt/trn_rl_repo" not in sys.path:
    sys.path.insert(0, "/opt/trn_rl_repo")

import concourse.bass as bass
import concourse.bacc as bacc
import concourse.mybir as mybir
import concourse.tile as tile
from concourse.bass_utils import run_bass_kernel_spmd

P = 128
B, C, NH, HD = 4, 1024, 16, 64
T_FULL = 2048
GC = 512          # per-core head-dim columns (8 heads x 64)
TB = 512          # free-dim tile width
NCC = C // P      # 8 contraction chunks for the qkv projection
F32 = mybir.dt.float32
F32R = mybir.dt.float32r

_NC_CACHE = {}


def _build(t_len: int, use_mask: bool) -> bass.Bass:
    from contextlib import ExitStack

    ntb = t_len // TB     # query blocks
    njc = t_len // P      # key chunks
    AOT = mybir.AluOpType
    ACTF = mybir.ActivationFunctionType

    nc = bacc.Bacc()
    xT = nc.dram_tensor("xT", [C, t_len], F32R, kind="ExternalInput")
    w_qk = nc.dram_tensor("w_qk", [C, 2 * GC], F32R, kind="ExternalInput")
    w_v = nc.dram_tensor("w_v", [C, GC], F32R, kind="ExternalInput")
    w_pr = nc.dram_tensor("w_pr", [GC, C], F32R, kind="ExternalInput")
    consts = nc.dram_tensor("consts", [P, 640], F32, kind="ExternalInput")
    masks = nc.dram_tensor("masks", [P, 4 * TB], F32R, kind="ExternalInput")
    onesr = nc.dram_tensor("onesr", [P, 64], F32R, kind="ExternalInput")
    vinit = nc.dram_tensor("vinit", [P, 4 * 193], F32R, kind="ExternalInput")
    bcsel = nc.dram_tensor("bcsel", [P, P], F32R, kind="ExternalInput")
    out = nc.dram_tensor("out", [t_len, C], F32, kind="ExternalOutput")

    with tile.TileContext(nc) as tc, ExitStack() as ctx:
        persist = ctx.enter_context(tc.tile_pool(name="persist", bufs=1))
        # Q^T | K^T rows (q rows pre-scaled by 1/8 with bias folded in)
        qkT = persist.tile([P, 2 * GC // P, t_len], F32R)
        vna = persist.tile([P, njc, 4 * 193], F32R)
        mk = persist.tile([P, 4, TB], F32R)
        onr = persist.tile([P, 64], F32R)
        bcs = persist.tile([P, P], F32R)
        cst = persist.tile([P, 640], F32)
        wpj = persist.tile([P, GC // P, C], F32R)

        nc.sync.dma_start(cst[:], consts[:])
        nc.sync.dma_start(mk[:], masks.rearrange("p (s f) -> p s f", s=4))
        nc.sync.dma_start(onr[:], onesr[:])
        nc.sync.dma_start(bcs[:], bcsel[:])
        for jc in range(njc):
            nc.sync.dma_start(vna[:, jc, :], vinit[:])
        nc.sync.dma_start(wpj[:], w_pr.rearrange("(o p) n -> p o n", p=P))

        xT_r = xT.rearrange("(o p) t -> p o t", p=P)
        wqk_r = w_qk.rearrange("(o p) m -> p o m", p=P)

        # ---------------- Phase 1: QKV projection ----------------
        with tc.tile_pool(name="ph1w", bufs=1) as ph1w, \
             tc.tile_pool(name="ph1", bufs=2) as ph1, \
             tc.tile_pool(name="wqs", bufs=3) as wqs, \
             tc.tile_pool(name="ps1", bufs=3, space="PSUM") as ps1:
            wv_t = ph1w.tile([P, NCC, GC], F32R, tag="wv")
            nc.sync.dma_start(wv_t[:], w_v.rearrange("(o p) n -> p o n", p=P))

            for tb in range(ntb):
                xt = ph1.tile([P, NCC, TB], F32R, tag="xt")
                nc.sync.dma_start(xt[:], xT_r[:, :, tb * TB:(tb + 1) * TB])

                # V in natural [t, d] layout
                for tsb in range(TB // P):
                    jc = tb * (TB // P) + tsb
                    ps = ps1.tile([P, TB], F32, tag="ps")
                    for cc in range(NCC):
                        nc.tensor.matmul(
                            ps[:],
                            lhsT=xt[:, cc, tsb * P:(tsb + 1) * P],
                            rhs=wv_t[:, cc, :],
                            start=(cc == 0), stop=(cc == NCC - 1),
                        )
                    vv = vna[:, jc, :].rearrange("p (pr c) -> p pr c", c=193)
                    pr_ps = ps[:].rearrange("p (pr two c) -> p pr two c", two=2, c=64)
                    pr_bv = cst[:, 88:600].rearrange("p (pr two c) -> p pr two c", two=2, c=64)
                    nc.vector.tensor_tensor(
                        vv[:, :, 0:64], pr_ps[:, :, 0, :], pr_bv[:, :, 0, :], AOT.add
                    )
                    nc.vector.tensor_tensor(
                        vv[:, :, 129:193], pr_ps[:, :, 1, :], pr_bv[:, :, 1, :], AOT.add
                    )
                    if use_mask:
                        nc.vector.tensor_scalar_mul(
                            vna[:, jc, :], vna[:, jc, :], cst[:, 64 + jc:65 + jc]
                        )

                # Q^T / K^T rows (transposed layout)
                for mb in range(2 * GC // P):
                    wq = wqs.tile([P, NCC, P], F32R, tag="wq")
                    nc.sync.dma_start(wq[:], wqk_r[:, :, mb * P:(mb + 1) * P])
                    ps = ps1.tile([P, TB], F32, tag="ps")
                    for cc in range(NCC):
                        nc.tensor.matmul(
                            ps[:],
                            lhsT=wq[:, cc, :],
                            rhs=xt[:, cc, :],
                            start=(cc == 0), stop=(cc == NCC - 1),
                        )
                    dst = qkT[:, mb, tb * TB:(tb + 1) * TB]
                    bias = cst[:, 80 + mb:81 + mb]
                    if mb < GC // P:
                        nc.vector.tensor_scalar(
                            dst, ps[:], bias, 0.125, AOT.add, AOT.mult
                        )
                    else:
                        nc.vector.tensor_scalar(
                            dst, ps[:], bias, None, AOT.add
                        )

        # ---------------- Phase 2: causal attention ----------------
        with tc.tile_pool(name="ph2", bufs=1) as ph2:
            yT = ph2.tile([P, GC // P, t_len], F32R)
            _phase2(nc, tc, qkT, vna, mk, bcs, yT, t_len, use_mask)
            _phase3(nc, tc, yT, wpj, out, t_len)
    nc.finalize()
    return nc


def _phase2(nc, tc, qkT, vna, mk, bcs, yT, t_len, use_mask):
    ntb = t_len // TB
    AOT = mybir.AluOpType
    ACTF = mybir.ActivationFunctionType
    with tc.tile_pool(name="att", bufs=3) as att, \
         tc.tile_pool(name="rts", bufs=2) as rts, \
         tc.tile_pool(name="sps", bufs=2, space="PSUM") as sps, \
         tc.tile_pool(name="pvs", bufs=2, space="PSUM") as pvs:
        for pr in range(4):            # head pair: heads (2pr, 2pr+1)
            qc, kc = pr, GC // P + pr  # qkT chunk index of Q rows / K rows
            for ib in range(ntb):
                # per-head PV accumulators, one PSUM bank each:
                #   pve: [y_e rows 0..63 | denom_e row 64]   (lhsT M=65, v_e + ones col)
                #   pvo: [junk 0..62 | denom_o 63 | y_o 64..127] (lhsT M=128 window)
                pve = pvs.tile([P, TB], F32, tag="pve")
                pvo = pvs.tile([P, TB], F32, tag="pvo")
                njb = 4 * ib + 4
                for jb in range(njb):
                    sp = sps.tile([P, 2, TB], F32, tag="sp")
                    for e in range(2):
                        po = 64 * e
                        nc.tensor.matmul(
                            sp[:, e, :],
                            lhsT=qkT[po:po + 64, kc, jb * P:(jb + 1) * P],
                            rhs=qkT[po:po + 64, qc, ib * TB:(ib + 1) * TB],
                            start=True, stop=True,
                            tile_position=(po, 0),
                        )
                    pt = att.tile([P, 2, TB], F32R, tag="pt")
                    nc.scalar.activation(pt[:], sp[:], ACTF.Exp)
                    s = jb - 4 * ib
                    if s >= 0:         # diagonal chunk: causal 0/1 mask
                        for e in range(2):
                            nc.vector.tensor_tensor(
                                pt[:, e, :], pt[:, e, :], mk[:, s, :], AOT.mult
                            )
                    nc.tensor.matmul(
                        pve[0:65, :],
                        lhsT=vna[:, jb, pr * 193:pr * 193 + 65],
                        rhs=pt[:, 0, :],
                        start=(jb == 0), stop=(jb == njb - 1),
                    )
                    nc.tensor.matmul(
                        pvo[:, :],
                        lhsT=vna[:, jb, pr * 193 + 65:pr * 193 + 193],
                        rhs=pt[:, 1, :],
                        start=(jb == 0), stop=(jb == njb - 1),
                    )
                # normalize: reciprocal of the denominators, broadcast across
                # 64 partitions with K=1 matmuls, then elementwise multiply
                rt = rts.tile([P, 2, TB], F32R, tag="rt")
                with nc.allow_low_precision(reason="fp32r operand prep"):
                    nc.vector.reciprocal(rt[64:65, 0, :], pve[64:65, :])
                    nc.vector.reciprocal(rt[0:1, 1, :], pvo[0:1, :])
                # pack recip_e next to recip_o (partitions 1 and 0); one K=2
                # matmul against the selector broadcasts both across partitions
                nc.sync.dma_start(rt[1:2, 1, :], rt[64:65, 0, :])
                rb = sps.tile([P, 2, TB], F32, tag="sp")
                nc.tensor.matmul(
                    rb[:, 0, :],
                    lhsT=bcs[0:2, :],
                    rhs=rt[0:2, 1, :],
                    start=True, stop=True,
                )
                ye = yT[0:64, pr, ib * TB:(ib + 1) * TB]
                yo = yT[64:128, pr, ib * TB:(ib + 1) * TB]
                nc.vector.tensor_copy(ye, pve[0:64, :])
                nc.vector.tensor_copy(yo, pvo[64:128, :])
                nc.vector.tensor_tensor(ye, ye, rb[0:64, 0, :], AOT.mult)
                nc.vector.tensor_tensor(yo, yo, rb[64:128, 0, :], AOT.mult)


def _phase3(nc, tc, yT, wpj, out, t_len):
    with tc.tile_pool(name="ps3", bufs=3, space="PSUM") as ps3, \
         tc.tile_pool(name="opl", bufs=3) as opl:
        for tsb in range(t_len // P):
            for nb in range(C // TB):
                ps = ps3.tile([P, TB], F32, tag="po")
                for dc in range(GC // P):
                    nc.tensor.matmul(
                        ps[:],
                        lhsT=yT[:, dc, tsb * P:(tsb + 1) * P],
                        rhs=wpj[:, dc, nb * TB:(nb + 1) * TB],
                        start=(dc == 0), stop=(dc == GC // P - 1),
                    )
                ot = opl.tile([P, TB], F32, tag="ot")
                nc.vector.tensor_copy(ot[:], ps[:])
                nc.sync.dma_start(
                    out[tsb * P:(tsb + 1) * P, nb * TB:(nb + 1) * TB], ot[:]
                )


def _causal_masks() -> np.ndarray:
    s = np.arange(4)[:, None, None]
    p = np.arange(P)[None, :, None]
    f = np.arange(TB)[None, None, :]
    m = (s * P + p <= f).astype(np.float32)          # [4, 128, 512]
    return np.ascontiguousarray(np.transpose(m, (1, 0, 2)).reshape(P, 4 * TB))


def _make_in_maps(x, W_attn, b_attn, W_proj, attention_mask, t_len):
    masks_arr = _causal_masks()
    bcsel_arr = np.zeros((P, P), np.float32)
    bcsel_arr[0, 64:128] = 1.0
    bcsel_arr[1, 0:64] = 1.0
    vinit = np.zeros((P, 4 * 193), np.float32)
    for prh in range(4):
        vinit[:, prh * 193 + 64] = 1.0
        vinit[:, prh * 193 + 65] = 1.0
    in_maps = []
    for core in range(8):
        b, g = core // 2, core % 2
        qcols = slice(g * GC, (g + 1) * GC)
        kcols = slice(C + g * GC, C + (g + 1) * GC)
        vcols = slice(2 * C + g * GC, 2 * C + (g + 1) * GC)

        xTn = np.ascontiguousarray(x[b].T.astype(np.float32))
        w_qk = np.ascontiguousarray(
            np.concatenate([W_attn[:, qcols], W_attn[:, kcols]], axis=1).astype(np.float32)
        )
        w_v = np.ascontiguousarray(W_attn[:, vcols].astype(np.float32))
        w_pr = np.ascontiguousarray(W_proj[g * GC:(g + 1) * GC, :].astype(np.float32))

        cst = np.zeros((P, 640), np.float32)
        cst[:, 0:64] = 1.0
        km = attention_mask[b].astype(np.float32).reshape(t_len // P, P).T  # [128, njc]
        cst[:, 64:64 + t_len // P] = km
        b_qk = np.concatenate([b_attn[qcols], b_attn[kcols]]).astype(np.float32)
        cst[:, 80:88] = b_qk.reshape(8, P).T
        cst[:, 88:600] = np.broadcast_to(b_attn[vcols].astype(np.float32), (P, GC))

        in_maps.append({
            "xT": xTn, "w_qk": w_qk, "w_v": w_v, "w_pr": w_pr,
            "consts": cst, "masks": masks_arr,
            "onesr": np.ones((P, 64), np.float32),
            "bcsel": bcsel_arr,
            "vinit": vinit,
        })
    return in_maps


def _run(x, W_attn, b_attn, W_proj, b_proj, attention_mask, trace=False):
    t_len = x.shape[1]
    use_mask = not bool(np.all(attention_mask != 0))
    key = (t_len, use_mask)
    if key not in _NC_CACHE:
        _NC_CACHE[key] = _build(t_len, use_mask)
    nc = _NC_CACHE[key]
    in_maps = _make_in_maps(x, W_attn, b_attn, W_proj, attention_mask, t_len)
    res = run_bass_kernel_spmd(nc, in_maps, list(range(8)), trace=trace)
    outs = [res.results[i]["out"] for i in range(8)]
    bp = b_proj.astype(np.float32)[None, :]
    y = np.stack([outs[2 * b] + outs[2 * b + 1] + bp for b in range(B)]).astype(np.float32)
    return y, res


def kernel(x, W_attn, b_attn, W_proj, b_proj, attention_mask):
    x = np.asarray(x, np.float32)
    W_attn = np.asarray(W_attn, np.float32)
    b_attn = np.asarray(b_attn, np.float32)
    W_proj = np.asarray(W_proj, np.float32)
    b_proj = np.asarray(b_proj, np.float32)
    attention_mask = np.asarray(attention_mask)
    y, _ = _run(x, W_attn, b_attn, W_proj, b_proj, attention_mask)
    return y
